# revision 14
# baseline (speedup 1.0000x reference)
"""NetVLAD forward kernel for 8 TRN2 NeuronCores (Bass/Tile).

Reference (per batch b of 32):
  s = x @ Wk + b         (1024, 64) logits;  softmax over k -> a
  v[d,k] = sum_n a[n,k] x[n,d] + (sum_n a[n,k]) * C[d,k]
  v /= ||v||_2 over d (per k);  out = flatten(v) / ||flatten(v)||_2

Sharding: data-parallel over batch B=32 across 8 cores (4 batches/core).
Wk, b, C replicated; no collectives; host concatenates outputs.

Key layout/precision tricks vs the f32-input baseline:
  - x is uploaded in BOTH layouts (natural [n,d] for the aggregation
    matmul's moving operand, pre-transposed [d,n] for the logits matmul)
    as fp8 e3m4 -> 4MB HBM/core, and ZERO on-chip transposes of x
    (the baseline burned 128 PE matmuls/core transposing x).
  - Wk is host-prescaled by 64 (fp8 denormal avoidance); undone for free
    by the Exp activation's scale=1/64.
  - the e-transpose back to [n,k] is a regular matmul against an extended
    identity [I | g0col | g1col]; the two extra columns compute the
    softmax denominators Z (scaled by 1/64) in the same instruction.
  - softmax normalization is folded into a (8 small [128,64] DVE muls)
    instead of into x (8 big [128,512] muls); a stored as 64*a in fp8.
    All scale factors are powers of two and cancel in the L2 norms.
  - the final v^T transposes run as bf16 regular matmuls (64-col streams).
Engines: PE = matmuls only (~8.7K cols/batch), ACT = Exp + PSUM copies,
DVE = reciprocals/scaling/norm tail, sync+scalar issue HWDGE DMAs.
"""

import sys

sys.path.insert(0, "/opt/trn_rl_repo")

from contextlib import ExitStack

import numpy as np

import concourse.bacc as bacc
import concourse.tile as tile
from concourse import mybir
from concourse.bass_utils import run_bass_kernel_spmd

F32 = mybir.dt.float32
BF16 = mybir.dt.bfloat16
AX = mybir.AxisListType
OP = mybir.AluOpType
ACTF = mybir.ActivationFunctionType

B_PER_CORE = 4  # 32 batches / 8 cores
N = 1024  # H*W pixels per batch
D = 512
K = 64
EPS = 1e-12
N_CORES = 8

USE_FP8 = True
XDT = mybir.dt.float8e3 if USE_FP8 else BF16  # x / Wk / a storage dtype
WS = 64.0 if USE_FP8 else 1.0  # host pre-scale on Wk (denormal avoidance)
AS = 64.0 if USE_FP8 else 1.0  # on-chip scale on a (denormal avoidance)


def build_kernel():
    nc = bacc.Bacc()
    # [p, 4b+j, n]: xT[d=128j+p, n] per batch
    xt_d = nc.declare_dram_parameter("xt", [128, 4 * B_PER_CORE, N], XDT, isOutput=False)
    # [p, 8b+i, d]: x[n=128i+p, d] per batch
    xn_d = nc.declare_dram_parameter("xn", [128, 8 * B_PER_CORE, D], XDT, isOutput=False)
    out = nc.declare_dram_parameter("out", [B_PER_CORE, D * K], F32, isOutput=True)
    wkb_d = nc.declare_dram_parameter("wkb", [128, 4, K], XDT, isOutput=False)  # WS*Wk [p,j,k]
    idext_d = nc.declare_dram_parameter("idext", [128, 130], BF16, isOutput=False)
    id64_d = nc.declare_dram_parameter("id64", [128, K], BF16, isOutput=False)
    ct2_d = nc.declare_dram_parameter("ct2", [128, D], BF16, isOutput=False)  # [C^T; C^T]
    b2_d = nc.declare_dram_parameter("b2", [128, 1], F32, isOutput=False)  # [b; b]

    with tile.TileContext(nc) as tc, ExitStack() as ctx:
        const = ctx.enter_context(tc.tile_pool(name="const", bufs=1))
        xtp = ctx.enter_context(tc.tile_pool(name="xtp", bufs=4))
        xnp = ctx.enter_context(tc.tile_pool(name="xnp", bufs=4))
        sbm = ctx.enter_context(tc.tile_pool(name="sbm", bufs=2))
        nrm = ctx.enter_context(tc.tile_pool(name="nrm", bufs=2))
        # PSUM: s2 + a2 + v2 + as1 = 7 banks; o reuses the s pool
        ps_s = ctx.enter_context(tc.tile_pool(name="ps_s", bufs=2, space="PSUM"))
        ps_a = ctx.enter_context(tc.tile_pool(name="ps_a", bufs=3, space="PSUM"))
        ps_v = ctx.enter_context(tc.tile_pool(name="ps_v", bufs=2, space="PSUM"))
        ps_o = ctx.enter_context(tc.tile_pool(name="ps_o", bufs=1, space="PSUM"))

        # ---- tiles for x (loads issued below, interleaved with consts) ----
        xts, xns = [], []
        for b in range(B_PER_CORE):
            xts.append(xtp.tile([128, 4, N], XDT, tag=f"xt{b}", name=f"xtt{b}"))
            xns.append(xnp.tile([128, 8, D], XDT, tag=f"xn{b}", name=f"xnt{b}"))
        wkb = const.tile([128, 4, K], XDT)
        idext = const.tile([128, 130], BF16)
        b2_sb = const.tile([128, 1], F32)
        id64 = const.tile([128, K], BF16)
        ct2 = const.tile([128, D], BF16)
        ones_col = const.tile([128, 1], XDT)
        nc.vector.memset(ones_col[:], 1.0)
        eps64_sb = const.tile([128, 1], F32)
        nc.vector.memset(eps64_sb[:], float(64 * EPS))

        # x loads first (big transfers), consts slotted between; b0's xT is
        # split by n-group so mm1(g=0) can start after the first 256KB
        nc.sync.dma_start(out=xts[0][:, :, 0:512], in_=xt_d[:, 0:4, 0:512])
        nc.scalar.dma_start(out=wkb[:], in_=wkb_d[:])
        nc.scalar.dma_start(out=xts[1][:], in_=xt_d[:, 4:8, :])
        nc.sync.dma_start(out=xts[0][:, :, 512:1024], in_=xt_d[:, 0:4, 512:1024])
        nc.scalar.dma_start(out=b2_sb[:], in_=b2_d[:])
        nc.sync.dma_start(out=xns[0][:], in_=xn_d[:, 0:8, :])
        nc.scalar.dma_start(out=idext[:], in_=idext_d[:])
        nc.scalar.dma_start(out=xns[1][:], in_=xn_d[:, 8:16, :])
        nc.sync.dma_start(out=xts[2][:], in_=xt_d[:, 8:12, :])
        nc.scalar.dma_start(out=xts[3][:], in_=xt_d[:, 12:16, :])
        nc.sync.dma_start(out=xns[2][:], in_=xn_d[:, 16:24, :])
        nc.scalar.dma_start(out=xns[3][:], in_=xn_d[:, 24:32, :])
        nc.scalar.dma_start(out=id64[:], in_=id64_d[:])
        nc.scalar.dma_start(out=ct2[:], in_=ct2_d[:])

        as2 = ps_o.tile([128, 2], F32, tag="as2")
        S_all = nrm.tile([128, 2], F32, tag="sall")

        e2t_all, a2t_all, v_all = {}, {}, {}

        def emit_mm1_exp(p):
            bpair = (2 * p, 2 * p + 1)
            for h, b in enumerate(bpair):
                s_ps = ps_s.tile([128, 512], F32, tag="s", name=f"s{b}")
                for j in range(4):
                    for g in range(2):
                        nc.tensor.matmul(
                            s_ps[K * g : K * (g + 1), :],
                            wkb[:, j, :],
                            xts[b][:, j, 512 * g : 512 * (g + 1)],
                            start=(j == 0),
                            stop=(j == 3),
                            skip_group_check=True,
                        )
                eT = sbm.tile([128, 512], BF16, tag="eT", name=f"eT{b}")
                nc.scalar.activation(
                    eT[:], s_ps[:], ACTF.Exp, bias=b2_sb[:], scale=1.0 / WS
                )
                e2t_all[b] = eT
                a2t_all[b] = sbm.tile([128, 4, 128], XDT, tag="a", name=f"a{b}")

        def emit_chunks_mm2(p):
            bpair = (2 * p, 2 * p + 1)
            # transpose e to [n,k]; Z/AS rides the extended identity cols
            for c in range(4):
                for h, b in enumerate(bpair):
                    a_ps = ps_a.tile([128, 130], F32, tag="aps")
                    nc.tensor.matmul(
                        a_ps[:],
                        e2t_all[b][:, 128 * c : 128 * (c + 1)],
                        idext[:],
                        start=True,
                        stop=True,
                        skip_group_check=True,
                    )
                    iv = sbm.tile([128, 2], F32, tag="iv")
                    nc.vector.reciprocal(iv[:], a_ps[:, 128:130])
                    for g in range(2):
                        nc.vector.tensor_scalar_mul(
                            a2t_all[b][:, c, K * g : K * (g + 1)],
                            a_ps[:, K * g : K * (g + 1)],
                            iv[:, g : g + 1],
                        )
            # mm2 + asum, h0/h1 interleaved -> PE pairs disjoint col-groups
            v_ps = ps_v.tile([128, 512], F32, tag="v", name=f"v{p}")
            v_all[p] = v_ps
            for c in range(4):
                for g in range(2):
                    i = 4 * g + c
                    for h, b in enumerate(bpair):
                        nc.tensor.matmul(
                            v_ps[K * h : K * (h + 1), :],
                            a2t_all[b][:, c, K * g : K * (g + 1)],
                            xns[b][:, i, :],
                            start=(c == 0 and g == 0),
                            stop=(c == 3 and g == 1),
                            skip_group_check=True,
                        )
            for c in range(4):
                for g in range(2):
                    for h, b in enumerate(bpair):
                        nc.tensor.matmul(
                            as2[K * h : K * (h + 1), p : p + 1],
                            a2t_all[b][:, c, K * g : K * (g + 1)],
                            ones_col[:],
                            start=(c == 0 and g == 0),
                            stop=(c == 3 and g == 1),
                            skip_group_check=True,
                        )

        def emit_tail(p):
            # vvb = C*asum + v (bf16); S = sum_d vvb^2; sc = 1/(8*sqrt(S+eps));
            # sc applied by streaming diag(sc) through the vT transposes
            asum = nrm.tile([128, 1], F32, tag="asum", name=f"asum{p}")
            nc.vector.tensor_copy(asum[:], as2[:, p : p + 1])
            vc = nrm.tile([128, D], F32, tag="vc", name=f"vc{p}")
            nc.vector.tensor_scalar_mul(vc[:], ct2[:], asum[:])
            vvb = nrm.tile([128, D], BF16, tag="vvb", name=f"vvb{p}")
            nc.vector.tensor_add(vvb[:], vc[:], v_all[p][:])
            sq = nrm.tile([128, D], F32, tag="sq", name=f"sq{p}")
            nc.vector.tensor_mul(sq[:], vvb[:], vvb[:])
            nc.vector.reduce_sum(S_all[:, p : p + 1], sq[:], axis=AX.X)
            q8 = nrm.tile([128, 1], F32, tag="q8", name=f"q8{p}")
            nc.scalar.activation(
                q8[:], S_all[:, p : p + 1], ACTF.Sqrt, bias=eps64_sb[:], scale=64.0
            )
            sc1 = nrm.tile([128, 1], F32, tag="sc1", name=f"sc1{p}")
            nc.vector.reciprocal(sc1[:], q8[:])
            dg = nrm.tile([128, K], BF16, tag="dg", name=f"dg{p}")
            nc.vector.tensor_scalar_mul(dg[:], id64[:], sc1[:])
            for hh in range(2):
                bb_i = 2 * p + hh
                o_ps = ps_s.tile([128, 4, K], F32, tag="s", name=f"o{bb_i}")
                for j in range(4):
                    nc.tensor.matmul(
                        o_ps[:, j, :],
                        vvb[K * hh : K * (hh + 1), j * 128 : (j + 1) * 128],
                        dg[K * hh : K * (hh + 1), :],
                        start=True,
                        stop=True,
                        skip_group_check=True,
                    )
                o_sb = nrm.tile([128, 4, K], F32, tag="osb", name=f"osb{bb_i}")
                if hh == 0:
                    nc.scalar.copy(o_sb[:], o_ps[:])
                else:
                    nc.vector.tensor_copy(o_sb[:], o_ps[:])
                nc.sync.dma_start(
                    out=out[bb_i].rearrange("(j p k) -> p j k", j=4, p=128, k=K),
                    in_=o_sb[:],
                )

        # pair-pipelined emission: pair0's output path is emitted after
        # pair1's exps so it executes under pair1's compute, and the ACT
        # queue sees all Exps before the first Sqrt (2 table loads total)
        emit_mm1_exp(0)
        emit_chunks_mm2(0)
        emit_mm1_exp(1)
        emit_tail(0)
        emit_chunks_mm2(1)
        emit_tail(1)

    nc.compile()
    return nc


_CACHED_NC = None


def _get_nc():
    global _CACHED_NC
    if _CACHED_NC is None:
        _CACHED_NC = build_kernel()
    return _CACHED_NC


def build_in_maps(x, Wk, b, C):
    import ml_dtypes

    XNP = ml_dtypes.float8_e3m4 if USE_FP8 else ml_dtypes.bfloat16
    B = x.shape[0]
    x2 = np.ascontiguousarray(x, dtype=np.float32).reshape(B, N, D)
    bpc = B // N_CORES
    Wkf = np.asarray(Wk, dtype=np.float32)
    Cf = np.asarray(C, dtype=np.float32)
    bf = np.asarray(b, dtype=np.float32).reshape(K)
    idext = np.zeros((128, 130), dtype=np.float32)
    idext[:, :128] = np.eye(128)
    idext[0:64, 128] = 1.0 / AS
    idext[64:128, 129] = 1.0 / AS
    consts = {
        "wkb": np.ascontiguousarray(
            (Wkf * WS).reshape(4, 128, K).transpose(1, 0, 2)
        ).astype(XNP),
        "idext": idext.astype(ml_dtypes.bfloat16),
        "id64": np.concatenate([np.eye(K), np.eye(K)], axis=0).astype(
            ml_dtypes.bfloat16
        ),
        "ct2": np.concatenate([Cf.T, Cf.T], axis=0).astype(ml_dtypes.bfloat16),
        "b2": np.concatenate([bf, bf]).reshape(128, 1),
    }
    in_maps = []
    for c in range(N_CORES):
        xc = x2[c * bpc : (c + 1) * bpc]  # [4, 1024, 512]
        xn = np.ascontiguousarray(
            xc.reshape(bpc, 8, 128, D).transpose(2, 0, 1, 3).reshape(128, 8 * bpc, D)
        ).astype(XNP)
        xt = np.ascontiguousarray(
            xc.transpose(2, 0, 1)
            .reshape(4, 128, bpc, N)
            .transpose(1, 2, 0, 3)
            .reshape(128, 4 * bpc, N)
        ).astype(XNP)
        in_maps.append({"xt": xt, "xn": xn, **consts})
    return in_maps


def kernel(x, Wk, b, C):
    """Full-input NetVLAD forward. x (32,32,32,512) f32 -> out (32, 32768) f32."""
    in_maps = build_in_maps(x, Wk, b, C)
    nc = _get_nc()
    res = run_bass_kernel_spmd(nc, in_maps, list(range(N_CORES)))
    return np.concatenate([res.results[c]["out"] for c in range(N_CORES)], axis=0)
